# revision 1
# baseline (speedup 1.0000x reference)
"""Trainium2 Bass kernel for BaseLUTLayer (probabilistic LUT node eval).

Math (per reference):
  x_eff = where(flip, 1 - x, x)                      # (B, IN)
  g[b,n,j] = x_eff[b, mapping[n,j]]                  # gather, (B, N, 6)
  out[b,n] = sum_k sigmoid(lut[n,k]) * prod_j (g_j if bit_j(k) else 1-g_j)

Evaluated on-device as a 6-level multilinear contraction per (b, n):
  level 0 folds the LSB of the 64-entry sigmoid table with per-node
  scalars (tensor_scalar FMA, per-partition scalar operands), levels 1-5
  are lerps V = U_even + a_j * (U_odd - U_even) done with tensor_tensor
  ops and a 0-stride broadcast AP for a_j.

Sharding: nodes split 8 ways (1024 nodes/core); batch replicated.
x and flip are host-transposed to (IN, B) so dma_gather (the SWDGE
embedding-lookup primitive) can fetch one 256-float row per (node, fanin)
index.  Per-core output is (1024, 256), host concatenates + transposes.
"""

import numpy as np

B = 256
IN = 8192
NN = 8192
FAN = 6
NPAT = 64
NCORES = 8
PT = 128  # nodes per tile (partition dim)

_CACHE = {}


def _build_nc(nl, b, inp, fp16=True):
    """Build + compile the SPMD Bass program for one core's slice.

    nl: nodes per core, b: batch (replicated), inp: input size.
    """
    import concourse.bacc as bacc
    import concourse.mybir as mybir
    from concourse.tile import TileContext
    from concourse._compat import get_trn_type

    dt = mybir.dt
    Alu = mybir.AluOpType
    Act = mybir.ActivationFunctionType

    nt = nl // PT
    n_idx = nl * FAN          # gather indices total
    n_idx_t = PT * FAN        # per tile (768)
    iw = n_idx // 16          # idx wrap columns

    nc = bacc.Bacc(
        get_trn_type() or "TRN2",
        target_bir_lowering=False,
        debug=False,
        num_devices=NCORES,
    )
    # merged gather table: per input row, 2*b bytes of fp16 x then b bytes of u8 flip
    rowb = 3 * b
    xfT = nc.dram_tensor("xfT", [inp, rowb], dt.uint8, kind="ExternalInput")
    lut = nc.dram_tensor("lut", [nl, NPAT], dt.float32, kind="ExternalInput")
    idx = nc.dram_tensor("idx", [128, iw], dt.int16, kind="ExternalInput")
    outT = nc.dram_tensor("outT", [nl, b], dt.float32, kind="ExternalOutput")

    cdt = dt.float16 if fp16 else dt.float32

    with TileContext(nc) as tc:
        with (
            tc.tile_pool(name="const", bufs=1) as cpool,
            tc.tile_pool(name="ld", bufs=2) as ld,
            tc.tile_pool(name="small", bufs=3) as sm,
            tc.tile_pool(name="work", bufs=2) as wk,
        ):
            idx_sb = cpool.tile([128, iw], dt.int16)
            nc.sync.dma_start(idx_sb[:, :], idx[:, :])

            for t in range(nt):
                # --- loads: one gather brings x (fp16) + flip (u8) rows ---
                g = ld.tile([128, FAN, rowb], dt.uint8, tag="g")
                nc.gpsimd.dma_gather(
                    g[:, :, :], xfT[:, :], idx_sb[:, t * (n_idx_t // 16):(t + 1) * (n_idx_t // 16)],
                    n_idx_t, n_idx_t, rowb,
                )
                xg = g[:, :, 0:2 * b].bitcast(dt.float16)
                fg = g[:, :, 2 * b:rowb]
                lut_t = ld.tile([128, NPAT], dt.float32, tag="lut")
                nc.sync.dma_start(lut_t[:, :], lut[t * PT:(t + 1) * PT, :])

                # --- per-node table prep (Pool: small 2-input subs) ---
                # sig[k] = sigmoid(lut[k]); d0[m] = sig[2m+1]-sig[2m]
                # dE[q] = sig[4q+2]-sig[4q]; dD[q] = d0[2q+1]-d0[2q]
                sig = sm.tile([128, NPAT], dt.float32, tag="sig")
                nc.scalar.activation(sig[:, :], lut_t[:, :], Act.Sigmoid)
                d0 = sm.tile([128, NPAT // 2], dt.float32, tag="d0")
                nc.gpsimd.tensor_sub(d0[:, :], sig[:, 1::2], sig[:, 0::2])
                dE = sm.tile([128, NPAT // 4], dt.float32, tag="dE")
                nc.gpsimd.tensor_sub(dE[:, :], sig[:, 2::4], sig[:, 0::4])
                dD = sm.tile([128, NPAT // 4], dt.float32, tag="dD")
                nc.gpsimd.tensor_sub(dD[:, :], d0[:, 1::2], d0[:, 0::2])

                # --- flip: x_eff = |f - x|  (exact for f in {0,1}) ---
                # fanin 0 first (short critical path into level 0), 1-5 after
                ff = sm.tile([128, FAN, b], cdt, tag="ff")
                nc.scalar.activation(ff[:, :, :], fg[:, :, :], Act.Copy)
                dfx = sm.tile([128, FAN, b], cdt, tag="dfx")
                xe = sm.tile([128, FAN, b], cdt, tag="xe")
                nc.vector.tensor_sub(dfx[:, 0, :], ff[:, 0, :], xg[:, 0, :])
                nc.vector.tensor_sub(dfx[:, 1:, :], ff[:, 1:, :], xg[:, 1:, :])
                nc.scalar.activation(xe[:, 0, :], dfx[:, 0, :], Act.Abs)
                nc.scalar.activation(xe[:, 1:, :], dfx[:, 1:, :], Act.Abs)

                # --- level 0+1a: Ue[q] = sig[4q] + a0*d0[2q]
                #                 D1[q] = dE[q] + a0*dD[q]
                # 32 per-partition-scalar FMAs split across ACT/Pool/DVE, in
                # 4 q-chunks; level 1b (V1 = Ue + a1*D1) issued per chunk so
                # DVE starts before the whole level-0 sweep finishes.
                a0 = xe[:, 0, :]
                a1c = xe[:, 1:2, :]
                Ue = wk.tile([128, 16, b], cdt, tag="Ue")
                D1 = wk.tile([128, 16, b], cdt, tag="D1")
                P1 = wk.tile([128, 16, b], cdt, tag="P1")
                V = wk.tile([128, 16, b], cdt, tag="V1")
                # per chunk of 4 q's: 8 jobs; DVE-heavy on the ramp tile,
                # ACT/Pool-heavy in steady state
                if t == 0:
                    homes = ["dve", "dve", "dve", "dve", "dve", "dve", "pool", "pool"]
                elif t == 1:
                    homes = ["act", "act", "act", "dve", "dve", "pool", "pool", "pool"]
                else:
                    homes = ["act", "act", "act", "act", "act", "pool", "pool", "pool"]
                for c4 in range(4):
                    for i in range(8):
                        q = c4 * 4 + i // 2
                        if i % 2 == 0:
                            dst, sc, bi = Ue[:, q, :], d0[:, 2 * q:2 * q + 1], sig[:, 4 * q:4 * q + 1]
                        else:
                            dst, sc, bi = D1[:, q, :], dD[:, q:q + 1], dE[:, q:q + 1]
                        h = homes[i]
                        if h == "act":
                            nc.scalar.activation(dst, a0, Act.Identity, scale=sc, bias=bi)
                        elif h == "pool":
                            nc.gpsimd.tensor_scalar(
                                out=dst, in0=a0, scalar1=sc, scalar2=bi,
                                op0=Alu.mult, op1=Alu.add,
                            )
                        else:
                            nc.vector.tensor_scalar(
                                out=dst, in0=a0, scalar1=sc, scalar2=bi,
                                op0=Alu.mult, op1=Alu.add,
                            )
                    qs = slice(c4 * 4, c4 * 4 + 4)
                    a1 = a1c.broadcast_to([128, 4, b])
                    nc.vector.tensor_mul(P1[:, qs, :], D1[:, qs, :], a1)
                    nc.vector.tensor_add(V[:, qs, :], P1[:, qs, :], Ue[:, qs, :])

                # --- levels 2..5: V = U_e + a_j*(U_o - U_e) ---
                for j in range(2, 6):
                    h = 32 >> j  # output pattern count
                    eng = nc.gpsimd if (j == 5 and t != nt - 1) else nc.vector
                    D = wk.tile([128, h, b], cdt, tag=f"D{j}")
                    eng.tensor_sub(D[:, :, :], V[:, 1::2, :], V[:, 0::2, :])
                    a = xe[:, j:j + 1, :].broadcast_to([128, h, b])
                    P = wk.tile([128, h, b], cdt, tag=f"P{j}")
                    eng.tensor_mul(P[:, :, :], D[:, :, :], a)
                    odt = dt.float32 if j == 5 else cdt
                    Vn = wk.tile([128, h, b], odt, tag=f"V{j}")
                    eng.tensor_add(Vn[:, :, :], P[:, :, :], V[:, 0::2, :])
                    V = Vn

                nc.sync.dma_start(outT[t * PT:(t + 1) * PT, :], V[:, 0, :])

    nc.compile()
    return nc


def _prep_core_inputs(x, lut_table, mapping, flip_mask, nl, b, inp, n_cores=NCORES):
    """Host-side layout prep (pure data movement): transpose + slice + index pack."""
    xf = np.empty((inp, 3 * b), np.uint8)                          # (IN, 3B)
    xf[:, :2 * b] = np.ascontiguousarray(x.T, dtype=np.float16).view(np.uint8)
    xf[:, 2 * b:] = np.ascontiguousarray(flip_mask.T).astype(np.uint8)
    nt = nl // PT
    in_maps = []
    for c in range(n_cores):
        sl = slice(c * nl, (c + 1) * nl)
        lut_c = np.ascontiguousarray(lut_table[sl], dtype=np.float32)
        m_c = np.asarray(mapping[sl])                              # (nl, 6) int32
        # gather order: j = (t*6+f)*128 + p  ->  m_c[t*128+p, f]
        order = m_c.reshape(nt, PT, FAN).transpose(0, 2, 1).reshape(-1)
        idx16 = order.astype(np.int16)
        wrapped = np.ascontiguousarray(idx16.reshape(-1, 16).T)    # (16, nl*6/16)
        idx_full = np.tile(wrapped, (8, 1))                        # (128, ...)
        in_maps.append({"xfT": xf, "lut": lut_c, "idx": idx_full})
    return in_maps


def _run(nc, in_maps, **kw):
    from concourse.bass_utils import run_bass_kernel_spmd

    last = None
    for attempt in range(3):
        try:
            return run_bass_kernel_spmd(nc, in_maps, list(range(NCORES)), **kw)
        except Exception as e:  # transient device errors happen on this fabric
            last = e
            if "UNRECOVERABLE" not in str(e) and "UNAVAILABLE" not in str(e):
                raise
    raise last


def kernel(x, lut_table, mapping, flip_mask):
    b, inp = x.shape
    nn = lut_table.shape[0]
    nl = nn // NCORES
    key = (nl, b, inp)
    if key not in _CACHE:
        _CACHE[key] = _build_nc(nl, b, inp)
    nc = _CACHE[key]
    in_maps = _prep_core_inputs(x, lut_table, mapping, flip_mask, nl, b, inp)
    res = _run(nc, in_maps)
    outT = np.concatenate([res.results[c]["outT"] for c in range(NCORES)], axis=0)
    return np.ascontiguousarray(outT.T, dtype=np.float32)

